# revision 14
# baseline (speedup 1.0000x reference)
"""CapsuleLayer dynamic-routing kernel for 8 trn2 NeuronCores.

Strategy: shard the I axis (2048 input capsules) 8 ways. Each core's W slice
(pre-transposed on host to a zero-padded (i,d)-on-partitions layout, bf16)
is streamed through the PE as the moving operand. Routing runs as 3 launches:
  A:  s0 partials = sum_i u_hat (one big K=(i,d) matmul chain)
  BC: given v_(r-1): recompute u_hat per 4-i tile in PSUM (row+col tiled
      K=16 matmuls), fused beta/softmax/weighted-s accumulation on DVE.
Host glue between launches: 8-way partial sums + squash (tiny numpy).
B, I, D = 64, 2048, 16; N, E = 32, 32; 8 cores, 256 i per core.
"""
import sys
for _p in ("/opt/trn_rl_repo", "/opt/trn_rl_repo/concourse"):
    if _p not in sys.path:
        sys.path.append(_p)  # append, not prepend: prepending breaks axon jax plugin
import numpy as np
import ml_dtypes

B, I, D = 64, 2048, 16
N, E = 32, 32
NC = 8
IC = I // NC          # 256 i per core
T4 = IC // 4          # 64 tiles of 4 i's
NE = N * E            # 1024

_cache = {}


def _build_kernel_A():
    import concourse.bass as bass
    import concourse.bacc as bacc
    from concourse import mybir
    from concourse.tile import TileContext

    nc = bacc.Bacc()
    w_in = nc.dram_tensor("wz", [T4, 128, NE], mybir.dt.bfloat16, kind="ExternalInput")
    x_in = nc.dram_tensor("xz", [T4, 128, B], mybir.dt.bfloat16, kind="ExternalInput")
    s_out = nc.dram_tensor("s0p", [B, NE], mybir.dt.float32, kind="ExternalOutput")

    with TileContext(nc) as tc:
        with (
            tc.tile_pool(name="w", bufs=1) as wp,
            tc.tile_pool(name="x", bufs=1) as xp,
            tc.tile_pool(name="ps", bufs=2, space="PSUM") as pp,
            tc.tile_pool(name="o", bufs=4) as op,
        ):
            wt = wp.tile([128, T4, NE], mybir.dt.bfloat16)
            xt = xp.tile([128, T4, B], mybir.dt.bfloat16)
            nc.gpsimd.dma_start(out=wt, in_=w_in.rearrange("c p f -> p c f"))
            nc.gpsimd.dma_start(out=xt, in_=x_in.rearrange("c p f -> p c f"))
            G = 4
            gsz = T4 // G
            parts = []
            for g in range(G):
                ps = pp.tile([B, NE], mybir.dt.float32)
                for j in range(gsz):
                    t = g * gsz + j
                    for k in range(2):
                        nc.tensor.matmul(
                            ps[:, k * 512:(k + 1) * 512], xt[:, t, :],
                            wt[:, t, k * 512:(k + 1) * 512],
                            start=(j == 0), stop=(j == gsz - 1),
                        )
                sb_g = op.tile([B, NE], mybir.dt.float32)
                nc.vector.tensor_copy(sb_g, ps)
                parts.append(sb_g)
            for g in range(1, G):
                nc.vector.tensor_add(parts[0], parts[0], parts[g])
            nc.sync.dma_start(out=s_out[:, :], in_=parts[0])
    nc.compile()
    return nc


def _build_kernel_BC():
    import concourse.bass as bass
    import concourse.bacc as bacc
    from concourse import mybir
    from concourse.tile import TileContext

    AX = mybir.AxisListType
    OP = mybir.AluOpType
    AF = mybir.ActivationFunctionType

    nc = bacc.Bacc()
    w_in = nc.dram_tensor("wz", [T4, 128, NE], mybir.dt.bfloat16, kind="ExternalInput")
    x_in = nc.dram_tensor("xz", [T4, 128, B], mybir.dt.bfloat16, kind="ExternalInput")
    v_in = nc.dram_tensor("vin", [128, NE], mybir.dt.float32, kind="ExternalInput")
    bp_in = nc.dram_tensor("bprev", [128, T4 * 64], mybir.dt.float32, kind="ExternalInput")
    bn_out = nc.dram_tensor("bnew", [128, T4 * 64], mybir.dt.float32, kind="ExternalOutput")
    s_out = nc.dram_tensor("spart", [128, NE], mybir.dt.float32, kind="ExternalOutput")

    with TileContext(nc) as tc:
        with (
            tc.tile_pool(name="w", bufs=1) as wp,
            tc.tile_pool(name="x", bufs=1) as xp,
            tc.tile_pool(name="ps", bufs=2, space="PSUM") as pp,
            tc.tile_pool(name="big", bufs=2) as bigp,
            tc.tile_pool(name="sm", bufs=4) as smp,
            tc.tile_pool(name="st", bufs=1) as stp,
        ):
            v_sb = stp.tile([128, NE], mybir.dt.float32)
            nc.sync.dma_start(out=v_sb, in_=v_in[:, :])
            bnew = stp.tile([128, T4 * 64], mybir.dt.float32)
            nc.sync.dma_start(out=bnew, in_=bp_in[:, :])
            s_acc = stp.tile([128, NE], mybir.dt.float32)
            nc.vector.memset(s_acc, 0.0)

            v_bc = bass.AP(tensor=v_sb.tensor, offset=v_sb.offset,
                           ap=[v_sb.ap[0], [0, 2], *v_sb.ap[1:]])

            wt = wp.tile([128, T4, NE], mybir.dt.bfloat16)
            xt = xp.tile([128, T4, B], mybir.dt.bfloat16)
            nc.gpsimd.dma_start(out=wt, in_=w_in.rearrange("c p f -> p c f"))
            nc.gpsimd.dma_start(out=xt, in_=x_in.rearrange("c p f -> p c f"))

            for t in range(T4):
                # u_hat for 4 i's: partitions (x*64+b), free (y, n, e)
                ups = pp.tile([128, 2 * NE], mybir.dt.float32)
                for it in range(4):
                    x_, y_ = it % 2, it // 2
                    for k in range(2):
                        nc.tensor.matmul(
                            ups[x_ * 64:(x_ + 1) * 64,
                                y_ * NE + k * 512: y_ * NE + (k + 1) * 512],
                            xt[it * 32: it * 32 + 16, t, :],
                            wt[it * 32: it * 32 + 16, t, k * 512:(k + 1) * 512],
                            start=True, stop=True,
                            tile_position=(it * 32, x_ * 64),
                        )
                # beta = sum_e u*v  -> [128, (y n)=64]
                prod = bigp.tile([128, 2 * NE], mybir.dt.float32)
                nc.vector.tensor_mul(prod, ups, v_bc)
                beta = smp.tile([128, 64], mybir.dt.float32)
                nc.vector.tensor_reduce(
                    out=beta, in_=prod.rearrange("p (yn e) -> p yn e", e=E),
                    axis=AX.X, op=OP.add)
                bslice = bnew[:, t * 64:(t + 1) * 64]
                nc.vector.tensor_add(bslice, bslice, beta)
                # softmax over n within each y
                b3 = bslice.rearrange("p (y n) -> p y n", y=2)
                mx = smp.tile([128, 2], mybir.dt.float32)
                nc.vector.tensor_reduce(out=mx, in_=b3, axis=AX.X, op=OP.max)
                mx_bc = bass.AP(tensor=mx.tensor, offset=mx.offset,
                                ap=[mx.ap[0], [1, 2], [0, N]])
                ex = smp.tile([128, 2, N], mybir.dt.float32)
                nc.vector.tensor_sub(ex, b3, mx_bc)
                nc.scalar.activation(ex, ex, AF.Exp)
                sm = smp.tile([128, 2], mybir.dt.float32)
                nc.vector.tensor_reduce(out=sm, in_=ex, axis=AX.X, op=OP.add)
                rc = smp.tile([128, 2], mybir.dt.float32)
                nc.vector.reciprocal(rc, sm)
                rc_bc = bass.AP(tensor=rc.tensor, offset=rc.offset,
                                ap=[rc.ap[0], [1, 2], [0, N]])
                c_t = smp.tile([128, 2, N], mybir.dt.float32)
                nc.vector.tensor_mul(c_t, ex, rc_bc)
                # s_acc += sum_y c*u
                c_bc = bass.AP(tensor=c_t.tensor, offset=c_t.offset,
                               ap=[c_t.ap[0], [N, 2], [1, N], [0, E]])
                prod2 = bigp.tile([128, 2 * NE], mybir.dt.float32)
                nc.vector.tensor_mul(
                    prod2.rearrange("p (y n e) -> p y n e", y=2, n=N), ups.rearrange("p (y n e) -> p y n e", y=2, n=N), c_bc)
                p2 = prod2.rearrange("p (y ne) -> p y ne", y=2)
                nc.vector.tensor_add(s_acc, s_acc, p2[:, 0, :])
                nc.vector.tensor_add(s_acc, s_acc, p2[:, 1, :])

            nc.sync.dma_start(out=bn_out[:, :], in_=bnew)
            nc.sync.dma_start(out=s_out[:, :], in_=s_acc)
    nc.compile()
    return nc


def _squash(s):
    s2 = np.sum(s * s, axis=-1, keepdims=True)
    return (s2 / (1.0 + s2) / np.sqrt(s2 + 1e-7)) * s


def _prep(inputs, W):
    bf16 = ml_dtypes.bfloat16
    wz, xz = [], []
    for k in range(NC):
        sl = slice(k * IC, (k + 1) * IC)
        Wk = W[0, sl]                                  # [256, N, D, E]
        a = Wk.transpose(0, 2, 1, 3).reshape(T4, 4, D, NE)
        wpad = np.zeros((T4, 4, 32, NE), np.float32)
        wpad[:, :, :D] = a
        wz.append(np.ascontiguousarray(wpad.reshape(T4, 128, NE)).astype(bf16))
        Xk = inputs[:, sl, :]                          # [B, 256, D]
        x = Xk.transpose(1, 2, 0).reshape(T4, 4, D, B)
        xpad = np.zeros((T4, 4, 32, B), np.float32)
        xpad[:, :, :D] = x
        xz.append(np.ascontiguousarray(xpad.reshape(T4, 128, B)).astype(bf16))
    return wz, xz


def kernel(inputs, W):
    from concourse.bass_utils import run_bass_kernel_spmd

    inputs = np.asarray(inputs, np.float32)
    W = np.asarray(W, np.float32)
    wz, xz = _prep(inputs, W)
    cores = list(range(NC))

    if "A" not in _cache:
        _cache["A"] = _build_kernel_A()
        _cache["BC"] = _build_kernel_BC()

    # launch A: s0 partials
    in_maps = [{"wz": wz[k], "xz": xz[k]} for k in cores]
    rA = run_bass_kernel_spmd(_cache["A"], in_maps, core_ids=cores)
    s0 = sum(r["s0p"] for r in rA.results) / float(N)
    v = _squash(s0.reshape(B, N, E)).astype(np.float32)

    bprev = [np.zeros((128, T4 * 64), np.float32) for _ in cores]
    for _r in range(2):
        vin = np.tile(v.reshape(B, NE), (2, 1)).astype(np.float32)
        in_maps = [{"wz": wz[k], "xz": xz[k], "vin": vin, "bprev": bprev[k]}
                   for k in cores]
        rBC = run_bass_kernel_spmd(_cache["BC"], in_maps, core_ids=cores)
        s = sum(r["spart"][:B] + r["spart"][B:] for r in rBC.results)
        v = _squash(s.reshape(B, N, E)).astype(np.float32)
        bprev = [r["bnew"] for r in rBC.results]

    return v.astype(np.float32)
